# revision 27
# baseline (speedup 1.0000x reference)
"""STFT kernel for Trainium2 (8 NeuronCores, batch-parallel).

Computes the equivalent of:
    xp = reflect_pad(x, 512)
    frames[b, f, n] = xp[b, 256*f + n] * window[n]      (f < 1025, n < 1024)
    spec = rfft(frames, axis=-1)                        -> [B, 1025, 513]
    out  = transpose(spec, (0, 2, 1))                   -> [B, 513, 1025] c64

Algorithm (radix-4 decimation-in-frequency over the hop structure):
with n = 256*j + r and k = c + 4*k2 (c = k mod 4), e^{-i*th*k*256*j} =
(-i)^(c*j) depends only on c, so

    spec[f, k] = sum_r e^{-i*th*k*r} * U_c[f, r],
    U_c[f, r]  = sum_j (-i)^(c*j) * w[256j+r] * Y[f+j, r]

where Y[g, r] = xp[256*g + r] and th = 2*pi/1024.  The four U_c (real U0,
U2; complex U1; U3 = conj(U1)) are built once per batch on DVE/GpSimd from
shifted views of the transposed hop matrix Y^T (fused multiply-add
`scalar_tensor_tensor` ops), and each frequency class c is then a short
TensorE matmul with contraction over r (256) instead of n (1024) — ~2.8x
fewer PE stream cycles than the direct windowed DFT.  Signs are folded into
the precomputed class matrices so no extra elementwise fixups are needed:

    P0 = w0*Y0   P1 = w1*Y1   (ts_mul)
    Q  = w2*Y2 + P0           R   = w3*Y3 + P1      (stt mult,add)
    U1rn = w2*Y2 - P0 (= -Re U1)   U1i = w3*Y3 - P1 (= Im U1)  (stt mult,sub)
    U0 = Q + R                U2  = Q - R           (tt)

    re0 =  C0*U0              im0 =  S0*U0              (C = cos, S = -sin)
    re2 =  C2*U2              im2 =  S2*U2
    re1 = -C1*U1rn - S1*U1i   im1 = -S1*U1rn + C1*U1i
    re3 = -C3*U1rn + S3*U1i   im3 = -S3*U1rn - C3*U1i
    nyquist (k=512): re = sum_r (-1)^r U0[f, r]  (M=1 matmul), im = 0

Matmul operands are fp16 (fp32 PSUM accumulation): full fp32 matmul is a
2-pass HI/LO operation on TRN2 and its weight loads cannot use FWL.

Batch dim (16) is sharded across the 8 cores, 2 batches each; no
cross-device communication.
"""

from contextlib import ExitStack

import numpy as np

import concourse.mybir as mybir
import concourse.tile as tile
from concourse import bacc
from concourse.bass_utils import run_bass_kernel_spmd

NFFT, HOP, PAD = 1024, 256, 512
B, T = 16, 262144
NCORES = 8
BC = B // NCORES                 # batches per core
G = (T + 2 * PAD) // HOP         # 1028 hop blocks per padded row
GP = G + 2                       # padded so the garbage tail frame is in-bounds
NF = (T + 2 * PAD - NFFT) // HOP + 1   # 1025 frames
NFC = NF + 1                     # 1026: computed frames incl one garbage tail
KF = NFFT // 2 + 1               # 513 one-sided freqs
# (f0, fn, valid): matmul frame chunks; the garbage frame is not stored.
CHUNKS = [(0, 474, 474), (474, 474, 474), (948, 78, 77)]
# class matrices M[idx]: [re-of-class terms..., im-of-class terms...]
#   c0: (0: C0) re, (1: S0) im          on U0
#   c2: (2: C2) re, (3: S2) im          on U2
#   c1: re = (4)*U1rn + (5)*U1i, im = (6)*U1rn + (7)*U1i
#   c3: re = (8)*U1rn + (9)*U1i, im = (10)*U1rn + (11)*U1i
NMAT = 12

_cache = {}

import ml_dtypes
DT16 = mybir.dt.float16
NP16 = np.float16


def _build():
    nc = bacc.Bacc(
        "TRN2", target_bir_lowering=False, debug=False, num_devices=NCORES
    )
    f32 = mybir.dt.float32
    f16 = DT16
    xt_d = nc.dram_tensor("xt", [BC, 256, GP], f16, kind="ExternalInput")
    xs_d = nc.dram_tensor("xts", [BC, 256, GP], f16, kind="ExternalInput")
    wm_d = nc.dram_tensor("wm", [128, NMAT, 2, 128], f16, kind="ExternalInput")
    nyq_d = nc.dram_tensor("nyq", [128, 2, 2], f16, kind="ExternalInput")
    wsc_d = nc.dram_tensor("wsc", [128, 8], f32, kind="ExternalInput")
    out_d = nc.dram_tensor("out", [BC, KF, 2 * NF], f32, kind="ExternalOutput")

    with tile.TileContext(nc) as tc, ExitStack() as ctx:
        consts = ctx.enter_context(tc.tile_pool(name="consts", bufs=1))
        xpool = ctx.enter_context(tc.tile_pool(name="x", bufs=2 * BC))
        upool = ctx.enter_context(tc.tile_pool(name="u", bufs=2))
        opool = ctx.enter_context(tc.tile_pool(name="o", bufs=6))
        ppool = ctx.enter_context(tc.tile_pool(name="psum", bufs=8, space="PSUM"))

        # ---- loads, ordered for earliest first matmul: tiny wsc, then
        # batch-0 input halves, then the c0/c2 class matrices, then the
        # rest; batch-1 prefetches last. ----
        xs = {}
        for b in range(BC):
            for h in range(2):
                xs[(b, h, 0)] = xpool.tile([128, GP], f16, name=f"x{b}{h}")
                xs[(b, h, 1)] = xpool.tile([128, GP], f16, name=f"xs{b}{h}")
        wsc = consts.tile([128, 8], f32)
        nc.sync.dma_start(wsc[:], wsc_d.ap())
        for h in range(2):
            nc.sync.dma_start(xs[(0, h, 0)][:], xt_d.ap()[0, 128 * h : 128 * (h + 1), :])
            nc.sync.dma_start(xs[(0, h, 1)][:], xs_d.ap()[0, 128 * h : 128 * (h + 1), :])
        wmA = consts.tile([128, 4, 2, 128], f16)
        nc.sync.dma_start(wmA[:], wm_d.ap()[:, 0:4])
        wmB = consts.tile([128, NMAT - 4, 2, 128], f16)
        nc.sync.dma_start(wmB[:], wm_d.ap()[:, 4:NMAT])
        nyqw = consts.tile([128, 2, 2], f16)
        nc.sync.dma_start(nyqw[:], nyq_d.ap())
        # prefetch batch-1 input, but not before the batch-0 critical loads
        # have the DMA engines to themselves (packet-level round-robin would
        # otherwise delay the first compute).
        with tc.tile_wait_until(0.008):
            for b in range(1, BC):
                for h in range(2):
                    nc.sync.dma_start(
                        xs[(b, h, 0)][:], xt_d.ap()[b, 128 * h : 128 * (h + 1), :]
                    )
                    nc.sync.dma_start(
                        xs[(b, h, 1)][:], xs_d.ap()[b, 128 * h : 128 * (h + 1), :]
                    )

        def wmat(mi):
            return wmA[:, mi] if mi < 4 else wmB[:, mi - 4]

        for b in range(BC):
            # ---- build U0, U2, U1rn, U1i per r-half on DVE (ts + tt only;
            # odd hop-shifts come from the pre-shifted copy xts so every
            # view is 4-byte aligned and hits the DVE fast modes) ----
            U = {}
            for h in range(2):
                # wsc[:, 2j+h] = w[256j + 128h + p]
                wj = lambda j: wsc[:, 2 * j + h : 2 * j + h + 1]
                src_ = lambda j: xs[(b, h, j & 1)][:, (j // 2) * 2 : (j // 2) * 2 + NFC]
                P = []
                for j in range(4):
                    p_ = upool.tile([128, NFC], f16, name=f"p{j}{h}")
                    nc.vector.tensor_scalar_mul(p_[:], src_(j), wj(j))
                    P.append(p_)
                q = upool.tile([128, NFC], f16, name=f"q{h}")
                nc.vector.tensor_add(q[:], P[0][:], P[2][:])
                r_ = upool.tile([128, NFC], f16, name=f"r{h}")
                nc.vector.tensor_add(r_[:], P[1][:], P[3][:])
                u0 = upool.tile([128, NFC], f16, name=f"u0{h}")
                nc.vector.tensor_add(u0[:], q[:], r_[:])
                u2 = upool.tile([128, NFC], f16, name=f"u2{h}")
                nc.vector.tensor_sub(u2[:], q[:], r_[:])
                u1rn = upool.tile([128, NFC], f16, name=f"u1rn{h}")
                nc.vector.tensor_sub(u1rn[:], P[2][:], P[0][:])
                u1i = upool.tile([128, NFC], f16, name=f"u1i{h}")
                nc.vector.tensor_sub(u1i[:], P[3][:], P[1][:])
                U[("u0", h)] = u0
                U[("u2", h)] = u2
                U[("u1rn", h)] = u1rn
                U[("u1i", h)] = u1i

            # ---- frequency classes: short matmuls over r (K=256) ----
            # (dst row start, [(mat, U) re-terms], [(mat, U) im-terms])
            classes = [
                (0, [(0, "u0")], [(1, "u0")]),
                (2, [(2, "u2")], [(3, "u2")]),
                (1, [(4, "u1rn"), (5, "u1i")], [(6, "u1rn"), (7, "u1i")]),
                (3, [(8, "u1rn"), (9, "u1i")], [(10, "u1rn"), (11, "u1i")]),
            ]
            for ci, (f0, fn, valid) in enumerate(CHUNKS):
                for c, re_terms, im_terms in classes:
                    ps = {}
                    for part, terms in (("re", re_terms), ("im", im_terms)):
                        p = ppool.tile([128, 512], f32, name="ps")[:, :fn]
                        nmm = 2 * len(terms)
                        i = 0
                        for mi, uname in terms:
                            for h in range(2):
                                nc.tensor.matmul(
                                    p,
                                    wmat(mi)[:, h, :],
                                    U[(uname, h)][:, f0 : f0 + fn],
                                    start=(i == 0),
                                    stop=(i == nmm - 1),
                                )
                                i += 1
                        ps[part] = p
                    ot = opool.tile([128, 2 * fn], f32, name="ot")
                    if c in (2, 3):
                        nc.scalar.copy(ot[:, 0::2], ps["re"])
                    else:
                        nc.vector.tensor_copy(ot[:, 0::2], ps["re"])
                    nc.scalar.copy(ot[:, 1::2], ps["im"])
                    nc.sync.dma_start(
                        out_d.ap()[b, c : 512 : 4, 2 * f0 : 2 * (f0 + valid)],
                        ot[:, : 2 * valid],
                    )
                # Nyquist row (k=512): re = sum_r (-1)^r U0, im = 0 exactly.
                pn = ppool.tile([128, 512], f32, name="ps")[:1, :fn]
                for h in range(2):
                    nc.tensor.matmul(
                        pn,
                        nyqw[:, h, :1],
                        U[("u0", h)][:, f0 : f0 + fn],
                        start=(h == 0),
                        stop=(h == 1),
                    )
                otn = opool.tile([1, 2 * fn], f32, name="otn")
                nc.vector.tensor_copy(otn[:1, 0::2], pn[:1, :])
                nc.scalar.mul(otn[:1, 1::2], pn[:1, :], 0.0)
                nc.sync.dma_start(
                    out_d.ap()[b, 512:513, 2 * f0 : 2 * (f0 + valid)],
                    otn[:1, : 2 * valid],
                )
    nc.compile()
    return nc


def _consts(window):
    w = np.asarray(window, np.float64)
    th = 2.0 * np.pi / NFFT
    r = np.arange(256, dtype=np.float64)[:, None]
    k2 = np.arange(128, dtype=np.float64)[None, :]

    def cs(c):
        ang = th * (c + 4.0 * k2) * r
        return np.cos(ang), -np.sin(ang)

    C0, S0 = cs(0)
    C1, S1 = cs(1)
    C2, S2 = cs(2)
    C3, S3 = cs(3)
    mats = [C0, S0, C2, S2, -C1, -S1, -S1, C1, -C3, S3, -S3, -C3]
    # [256(r), 128(k2)] -> [128(p), 2(h), 128], stacked -> [128, NMAT, 2, 128]
    wm = np.stack(
        [m.reshape(2, 128, 128).transpose(1, 0, 2) for m in mats], axis=1
    ).astype(NP16)
    wm = np.ascontiguousarray(wm)

    nyq = np.empty((128, 2, 2), np.float64)
    sign = ((-1.0) ** np.arange(128))[:, None]
    nyq[:, :, 0] = sign  # (-1)^r = (-1)^p for r = 128h + p
    nyq[:, :, 1] = 0.0   # padding column (unused)
    nyq = nyq.astype(NP16)

    # wsc[p, 2j+h] = w[256j + 128h + p]
    wsc = np.ascontiguousarray(
        w.reshape(4, 2, 128).transpose(2, 0, 1).reshape(128, 8), dtype=np.float32
    )
    return wm, nyq, wsc


def prep_inputs(x, window):
    """Host-side shard/layout prep: per-core input maps."""
    xp = np.pad(np.asarray(x, np.float32), ((0, 0), (PAD, PAD)), mode="reflect")
    xt = np.zeros((B, HOP, GP), NP16)
    xt[:, :, :G] = xp.reshape(B, G, HOP).transpose(0, 2, 1)
    xts = np.zeros((B, HOP, GP), NP16)   # shifted one hop left
    xts[:, :, : G - 1] = xt[:, :, 1:G]
    wm, nyq, wsc = _consts(window)
    return [
        {
            "xt": xt[i * BC : (i + 1) * BC],
            "xts": xts[i * BC : (i + 1) * BC],
            "wm": wm,
            "nyq": nyq,
            "wsc": wsc,
        }
        for i in range(NCORES)
    ]


def get_nc():
    nc = _cache.get("nc")
    if nc is None:
        nc = _build()
        _cache["nc"] = nc
    return nc


def kernel(x, window, _trace=False, _trace_kwargs=None):
    nc = get_nc()
    in_maps = prep_inputs(x, window)
    res = run_bass_kernel_spmd(
        nc, in_maps, list(range(NCORES)), trace=_trace, **(_trace_kwargs or {})
    )
    _cache["last_results"] = res
    out = np.concatenate([r["out"] for r in res.results], axis=0)
    return np.ascontiguousarray(out).view(np.complex64)
